# revision 38
# baseline (speedup 1.0000x reference)
"""Trainium2 Bass kernel for nn_CMAModel (control-fused memory attention).

Math (reference):
  q  = x @ Wq.T + ctrl @ Wc.T                  [B,T,C]
  kv = [x; fwd_mem; rev_mem]                   [B,S,C], S = T+M+R = 5440
  k  = kv @ Wk.T ; v = kv @ Wv.T
  per head h (D=128): scores = q_h k_h^T / sqrt(D), causal mask on the
  local T block only; w = softmax(scores); out_h = w_loc v_loc + gate_h *
  (w_mem v_mem); gate = sigmoid(q @ Wg.T + bg); y = concat(out_h) @ Wo.T

Sharding (8 cores, SPMD — one program, per-core behavior via input data):
  core = b*4 + g  (b = batch, g = group 0..3).  24 units of (b, head,
  T-half).  Each core runs 3 "slots": slots 0,1 = both halves of a
  "pair" head, slot 2 = one half of a "single" head (shared with the
  neighbor core).  Per batch:
    g=0: pair h0, single (h1, half A)     g=1: pair h2, single (h1, B)
    g=2: pair h3, single (h4, half A)     g=3: pair h5, single (h4, B)
  K/V are computed on-device per head-cache (cache0 = pair head,
  cache1 = single head) from the core's batch kv, column-sliced weights.

Layouts: everything feature-major ([C, tokens]) so all matmuls are
  natural (lhsT = transposed weights supplied by the host; no on-device
  transposes).  Attention uses scoresT [s, t]: softmax denominators are
  per-t sums over the s (partition) axis, accumulated per-partition into
  a running R on DVE; the final cross-partition sum AND its broadcast to
  all 128 partitions happen in ONE matmul (all-ones [128,128] @ R).
  Causal masking is (iota >= thr) with host-supplied per-partition
  thresholds — fully data-driven, identical control flow on all cores.

Fused-query gate / ctrl bias are tiny host-side precomputes:
  wfT = (Wg_h @ Wq) per slot, qbs = (Wc @ ctrl) slice, gb3 = gate bias.

Output: per-slot out-projection partials y_p = Wo[:, h-slice].T-free
  contribution [768, 1024] in bf16; the host sums the 6 head partials
  per (batch, half) and transposes — the standard row-parallel unshard.
"""

import numpy as np

B, T, C, H, M, R = 2, 2048, 768, 6, 3072, 320
D = C // H          # 128
S = T + M + R       # 5440
P = 128
NT = (S + P - 1) // P          # 43 s-tiles (last has 64 rows)
NLOC = T // P                  # 16 local s-tiles
NCT = C // P                   # 6 feature tiles
THALF = T // 2                 # 1024
NCH = THALF // 512             # 2 chunks of 512 per half
DSCALE = float(D) ** -0.5

# per-batch slot maps: (pair_head, single_head, single_half) per group
GROUP_MAP = [(0, 1, 0), (2, 1, 1), (3, 4, 0), (5, 4, 1)]


def slot_units(g):
    hp, hs, hsh = GROUP_MAP[g]
    return [(hp, 0), (hp, 1), (hs, hsh)]


def _kchunks():
    out = []
    off = 0
    while off < S:
        w = min(512, S - off)
        out.append((off, w))
        off += w
    return out


KCH = _kchunks()               # 10x512 + 320


def build_nc(use_f32r=True, debug=False, att_bf16=True):
    import concourse.mybir as mybir
    import concourse.tile as tile
    from concourse import bacc

    f32 = mybir.dt.float32
    f32r = mybir.dt.float32r if use_f32r else f32
    adt = mybir.dt.bfloat16 if att_bf16 else f32r
    AF = mybir.ActivationFunctionType
    OP = mybir.AluOpType

    def mm(psum, lhsT, rhs, start=True, stop=True):
        nc.tensor.matmul(psum, lhsT, rhs, start=start, stop=stop)

    nc = bacc.Bacc("TRN2", target_bir_lowering=False, debug=False,
                   num_devices=8)

    dram = {}
    for name, shape in [
        ("kvT", [C, S]),            # batch kv, transposed
        ("xqT", [C, 3 * THALF]),    # per-slot x columns, transposed
        ("wqT", [C, 3 * P]),        # per-slot Wq head-rows, transposed
        ("wkT0", [C, P]),           # pair-head Wk rows, transposed
        ("wkT1", [C, P]),           # single-head Wk rows, transposed
        ("wvT2", [C, 2 * P]),       # [pair | single] Wv rows, transposed
        ("woT", [P, 3 * C]),        # per-slot Wo head-cols, transposed
        ("wfT", [C, 3]),            # per-slot fused gate weights (Wg_h@Wq)
        ("qbs", [P, 3]),            # per-slot q bias column (Wc@ctrl slice)
        ("gb3", [1, 3]),            # per-slot full gate bias
        ("ones_r", [1, P]),         # ones row (f32 bcast stationary)
        ("bandm", [P, 2 * 3072]),   # fp16 causal bias bands (slots01|slot2)
        ("trineg", [P, P]),         # fp16 -M lower-tri (bias stationary)
    ]:
        dt_ = f32
        if name in ("kvT", "xqT", "wqT", "wkT0", "wkT1", "wvT2", "wfT"):
            dt_ = mybir.dt.bfloat16
        if name == "woT":
            dt_ = f32 if att_bf16 else f32r
        if name in ("bandm", "trineg"):
            dt_ = mybir.dt.float16
        dram[name] = nc.dram_tensor(name, shape, dt_, kind="ExternalInput")
    yp = nc.dram_tensor("yp", [3 * C, THALF], mybir.dt.bfloat16,
                        kind="ExternalOutput")
    dbg = {}
    if debug:
        for name, shape in [("d_q", [P, 3 * THALF]), ("d_gate", [1, 3 * THALF]),
                            ("d_kh0", [P, 1024]), ("d_vh", [P, 512]),
                            ("d_att", [P, 3 * THALF])]:
            dbg[name] = nc.dram_tensor(name, shape, f32,
                                       kind="ExternalOutput")

    from contextlib import ExitStack

    with tile.TileContext(nc) as tc, ExitStack() as _ctx:
        consts = _ctx.enter_context(tc.tile_pool(name="consts", bufs=1))
        # ---- K/V weights first (single rearranged DMAs — keep the
        # gpsimd queue short so the kv stream starts immediately) ----
        wk0 = consts.tile([P, NCT, P], adt)
        wk1 = consts.tile([P, NCT, P], adt)
        wv2 = consts.tile([P, NCT, 2 * P], adt)
        nc.gpsimd.dma_start(out=wk0[:], in_=dram["wkT0"][:, :].rearrange(
            "(a p) d -> p a d", p=P))
        nc.gpsimd.dma_start(out=wk1[:], in_=dram["wkT1"][:, :].rearrange(
            "(a p) d -> p a d", p=P))
        nc.gpsimd.dma_start(out=wv2[:], in_=dram["wvT2"][:, :].rearrange(
            "(a p) d -> p a d", p=P))
        ones_row = consts.tile([1, P], f32)
        nc.sync.dma_start(out=ones_row[:], in_=dram["ones_r"][:, :])
        ones_sq = consts.tile([P, P], mybir.dt.float16)
        nc.vector.memset(ones_sq[:], 1.0)

        # ---- phase 2: K/V projections into SBUF caches; q inputs and
        # weights stream in alongside (interleaved, both queues) ----
        f16 = mybir.dt.float16
        kh0 = consts.tile([P, S], adt)
        kh1 = consts.tile([P, S], adt)
        vh = consts.tile([P, NT, 2 * P], f16)
        xq_all = consts.tile([P, NCT, 3 * THALF], adt)
        xq_parts = [(ct, c0) for ct in range(NCT)
                    for c0 in range(0, 3 * THALF, 512)]  # 36 x [128,512]
        with tc.tile_pool(name="kvp", bufs=8) as kvp, \
             tc.tile_pool(name="kvps", bufs=1, space="PSUM") as kvps:
            for sc, (off, w) in enumerate(KCH):
                pk0 = kvps.tile([P, 512], f32, tag="k0", bufs=2)
                pk1 = kvps.tile([P, 512], f32, tag="k1", bufs=2)
                subs = []
                o2 = off
                while o2 < off + w:
                    subs.append((o2 - off, min(P, off + w - o2)))
                    o2 += P
                pv = [kvps.tile([P, 2 * P], f32, tag=f"v{si}",
                                name=f"pv{si}", bufs=1)
                      for si in range(len(subs))]
                j0 = off // P
                for ct in range(NCT):
                    kv_t = kvp.tile([P, 512], adt, tag="kv")
                    eng = nc.sync if (sc * NCT + ct) % 2 == 0 else nc.gpsimd
                    eng.dma_start(
                        out=kv_t[:, :w],
                        in_=dram["kvT"][ct * P:(ct + 1) * P, off:off + w])
                    mm(pk0[:, :w], wk0[:, ct, :], kv_t[:, :w],
                       start=(ct == 0), stop=(ct == NCT - 1))
                    mm(pk1[:, :w], wk1[:, ct, :], kv_t[:, :w],
                       start=(ct == 0), stop=(ct == NCT - 1))
                    for si, (so, sw) in enumerate(subs):
                        mm(pv[si][:sw, :], kv_t[:, so:so + sw],
                           wv2[:, ct, :],
                           start=(ct == 0), stop=(ct == NCT - 1))
                        if ct == NCT - 1:
                            # drain each V sub right after its stop so
                            # the next chunk's V matmuls aren't blocked
                            nc.vector.tensor_copy(out=vh[:sw, j0 + si, :],
                                                  in_=pv[si][:sw, :])
                nc.vector.tensor_copy(out=kh0[:, off:off + w],
                                      in_=pk0[:, :w])
                nc.vector.tensor_copy(out=kh1[:, off:off + w],
                                      in_=pk1[:, :w])
                # x columns for q-proj: 4 pieces per chunk from sc=2,
                # alternating queues, so neither engine backs up
                if sc >= 2:
                    for pi in range(4):
                        k4 = (sc - 2) * 4 + pi
                        if k4 < len(xq_parts):
                            ct, c0 = xq_parts[k4]
                            eng = nc.sync if k4 % 2 == 0 else nc.gpsimd
                            eng.dma_start(
                                out=xq_all[:, ct, c0:c0 + 512],
                                in_=dram["xqT"][ct * P:(ct + 1) * P,
                                                c0:c0 + 512])

        # ---- remaining constants (after the kv stream is queued) ----
        wqt = consts.tile([P, NCT, 3 * P], adt)
        wfT3 = consts.tile([P, NCT, 3], adt)
        nc.gpsimd.dma_start(out=wqt[:], in_=dram["wqT"][:, :].rearrange(
            "(a p) d -> p a d", p=P))
        nc.gpsimd.dma_start(out=wfT3[:], in_=dram["wfT"][:, :].rearrange(
            "(a p) d -> p a d", p=P))
        qbs = consts.tile([P, 3], f32)
        nc.gpsimd.dma_start(out=qbs[:], in_=dram["qbs"][:, :])
        gb3 = consts.tile([1, 3], f32)
        nc.gpsimd.dma_start(out=gb3[:], in_=dram["gb3"][:, :])
        wot = consts.tile([P, 3 * C], adt)
        if att_bf16:
            nc.gpsimd.dma_start(out=wot[:], in_=dram["woT"][:, :])
        else:
            nc.sync.dma_start(out=wot[:], in_=dram["woT"][:, :])
        bandm = consts.tile([P, 2, 3072], f16)
        nc.gpsimd.dma_start(out=bandm[:],
                            in_=dram["bandm"][:, :].rearrange(
                                "p (a b) -> p a b", a=2))
        trineg = consts.tile([P, P], f16)
        nc.gpsimd.dma_start(out=trineg[:], in_=dram["trineg"][:, :])

        # ---- phase 3: q projection + gate (all inputs SBUF-resident) ----
        qsb = consts.tile([P, 3, THALF], adt)
        gate = consts.tile([1, 3, THALF], f32)
        with tc.tile_pool(name="qps", bufs=1, space="PSUM") as qps:
            for k in range(3):
                for ch in range(NCH):
                    pq = qps.tile([P, 512], f32, tag="q", bufs=3)
                    pg = qps.tile([1, 512], f32, tag="g", bufs=3)
                    for ct in range(NCT):
                        xs = xq_all[:, ct, k * THALF + ch * 512:
                                    k * THALF + (ch + 1) * 512]
                        mm(pq[:], wqt[:, ct, k * P:(k + 1) * P], xs,
                           start=(ct == 0), stop=(ct == NCT - 1))
                        mm(pg[:], wfT3[:, ct, k:k + 1], xs,
                           start=(ct == 0), stop=(ct == NCT - 1))
                    nc.vector.tensor_scalar_add(
                        qsb[:, k, ch * 512:(ch + 1) * 512], pq[:],
                        qbs[:, k:k + 1])
                    nc.scalar.activation(
                        gate[0:1, k, ch * 512:(ch + 1) * 512], pg[:],
                        AF.Sigmoid, bias=gb3[0:1, k:k + 1], scale=1.0)

        if debug:
            nc.gpsimd.dma_start(out=dbg["d_q"][:, :],
                               in_=qsb[:].rearrange("p a b -> p (a b)"))
            nc.sync.dma_start(out=dbg["d_gate"][0:1, :],
                              in_=gate[:].rearrange("p a b -> p (a b)"))
            nc.gpsimd.dma_start(out=dbg["d_kh0"][:, :], in_=kh0[:, 0:1024])
            nc.gpsimd.dma_start(out=dbg["d_vh"][:, :],
                               in_=vh[:, 0:2, :].rearrange("p a b -> p (a b)"))
        # ---- phase 4: attention + output projection, per slot ----
        # Per slot: one s-tile does 2 N=512 scoresT matmuls into one
        # 2-bank psum tile, one 1024-wide exp, a causal mask multiply on
        # diagonal tiles (precomputed mask tiles for slots 0/1 whose
        # halves are compile-time; threshold-generated for slot 2), a
        # single fp16 running-sum add (softmax denominator), and two AV
        # accumulations.  The slot-end normalization/out-projection chain
        # is DEFERRED into the next slot's j-loop so the PE never idles
        # across slot boundaries (keeps HAM warm).
        with tc.tile_pool(name="att", bufs=2) as att_pool, \
             tc.tile_pool(name="ep", bufs=15) as ep, \
             tc.tile_pool(name="vec", bufs=3) as vec, \
             tc.tile_pool(name="cmb", bufs=1) as cmb, \
             tc.tile_pool(name="ysb", bufs=3) as ysb, \
             tc.tile_pool(name="aps", bufs=1, space="PSUM") as aps:
            finalize_prev = None

            def make_finalize(k, Rt, Lsb, Msb):
                # psum comes from the AV-region tags ("mm"/"ll") which
                # are free early in the next slot's loop — NEVER from
                # the "sc" tag, whose 2-buf rotation would stall the
                # next slot's scores->exp pipeline behind this chain.
                # Emission is SPLIT into pieces consumed one per j-loop
                # iteration so the dependent out-proj matmuls land in
                # the PE stream after the DVE chain has had time to run.
                state = {}

                def fin_early():
                    # denominator: sum over partitions AND broadcast to
                    # all 128 rows in one all-ones matmul; ONE wide
                    # reciprocal (DVE recip has a ~3.3us fixed cost).
                    # Runs at the END of slot k while DVE is quiet —
                    # ready long before the out-proj needs it.
                    prb = aps.tile([P, NCH, 512], f32, tag="mm",
                                   name="prb")
                    for ch in range(NCH):
                        mm(prb[:, ch, :], ones_sq[:],
                           Rt[:, ch * 512:(ch + 1) * 512])
                    rb = cmb.tile([P, NCH, 512], f32r, tag="rb")
                    with nc.allow_low_precision(reason="f32r norm"):
                        nc.vector.reciprocal(
                            rb[:].rearrange("p a b -> p (a b)"),
                            prb[:].rearrange("p a b -> p (a b)"))
                    state['rb'] = rb

                def fin_head():
                    # UNNORMALIZED attb' = Lsb + gate*Msb (bf16); the
                    # 1/den scale is folded into the yt output multiply
                    attb = att_pool.tile([P, NCH, 512], adt, tag="attb")
                    pgb = aps.tile([P, NCH, 512], f32, tag="mm",
                                   name="pgb")
                    for ch in range(NCH):
                        mm(pgb[:, ch, :], ones_row[:],
                           gate[0:1, k, ch * 512:(ch + 1) * 512])
                    for ch in range(NCH):
                        t2 = cmb.tile([P, 512], f32, tag="t2")
                        nc.vector.tensor_tensor(t2[:], Msb[:, ch, :],
                                                pgb[:, ch, :], OP.mult)
                        with nc.allow_low_precision(reason="bf16 att"):
                            nc.vector.tensor_tensor(attb[:, ch, :], t2[:],
                                                    Lsb[:, ch, :], OP.add)
                    state['attb'] = attb

                def fin_py(ot):
                    py = aps.tile([P, NCH, 512], f32, tag="ll",
                                  name=f"py{ot}")
                    for ch in range(NCH):
                        mm(py[:, ch, :],
                           wot[:, k * C + ot * P:k * C + (ot + 1) * P],
                           state['attb'][:, ch, :])
                    yt = ysb.tile([P, NCH, 512], mybir.dt.bfloat16,
                                  tag="y")
                    with nc.allow_low_precision(reason="bf16 out"):
                        nc.vector.tensor_tensor(
                            yt[:].rearrange("p a b -> p (a b)"),
                            py[:].rearrange("p a b -> p (a b)"),
                            state['rb'][:].rearrange("p a b -> p (a b)"),
                            OP.mult)
                    nc.sync.dma_start(
                        out=yp[k * C + ot * P:k * C + (ot + 1) * P, :],
                        in_=yt[:].rearrange("p a b -> p (a b)"))

                return [fin_early, fin_head] + \
                    [lambda ot=ot: fin_py(ot) for ot in range(NCT)]

            for k in range(3):
                kh = kh0 if k < 2 else kh1
                voff = 0 if k < 2 else P
                loc_end = 8 if k == 0 else NLOC
                msk_lo = {0: 0, 1: 8, 2: 0}[k]
                # locals first, then memory: the "mm" (memory-AV) psum
                # banks stay free until idx ~16+lag, giving the deferred
                # finalize of the previous slot its psum without ever
                # touching the scores rotation
                js = list(range(loc_end)) + list(range(NLOC, NT))
                Rt = vec.tile([P, THALF], mybir.dt.float16, tag="R")
                Lsb = att_pool.tile([P, NCH, 512], f32, tag="Lsb")
                Msb = att_pool.tile([P, NCH, 512], f32, tag="Msb")
                qrhs = qsb[:, k, :]
                pacc = {}
                Et = {}
                pend = []

                def emit_av(j, k=k, voff=voff, loc_end=loc_end, pacc=pacc,
                            Et=Et):
                    spn = min(P, S - j * P)
                    E2 = Et.pop(j)
                    reg = 'l' if j < NLOC else 'm'
                    first = j == 0 or j == NLOC
                    last = j == loc_end - 1 or j == NT - 1
                    if first:
                        pacc[reg] = aps.tile(
                            [P, NCH, 512], f32,
                            tag=("ll" if reg == 'l' else "mm"),
                            name=f"pacc{reg}")
                    for ch in range(NCH):
                        mm(pacc[reg][:, ch, :], vh[:spn, j, voff:voff + P],
                           E2[:spn, ch * 512:(ch + 1) * 512],
                           start=first, stop=last)

                for idx, j in enumerate(js):
                    # one finalize piece every other iteration from idx 8:
                    # head (den/gate/attb chain) at 8, out-proj pieces at
                    # 10,12,... so each py matmul finds attb/yt ready
                    if idx >= 8 and idx % 2 == 0 and finalize_prev:
                        finalize_prev.pop(0)()
                    spn = min(P, S - j * P)
                    masked = msk_lo <= j < loc_end
                    ps = aps.tile([P, NCH, 512], f32, tag="sc", bufs=2)
                    for ch in range(NCH):
                        mm(ps[:spn, ch, :], kh[:, j * P:j * P + spn],
                           qrhs[:, ch * 512:(ch + 1) * 512],
                           start=True, stop=not masked)
                    if masked:
                        # additive causal bias: -M * count(t < s) via
                        # (-M*lower_tri) @ shifted 0/1 band; per-core
                        # half-offset is baked into the band data
                        bsl = 1 if k == 2 else 0
                        u0 = 1920 - 128 * (j - msk_lo)
                        for ch in range(NCH):
                            mm(ps[:spn, ch, :], trineg[:, :spn],
                               bandm[:, bsl, u0 + ch * 512:
                                     u0 + (ch + 1) * 512],
                               start=False, stop=True)
                    E2 = ep.tile([P, THALF], f16, tag="E")
                    nc.scalar.activation(E2[:spn], ps[:spn].rearrange(
                        "p a b -> p (a b)"), AF.Exp, scale=DSCALE)
                    if idx == 0:
                        nc.vector.tensor_copy(out=Rt[:, :], in_=E2[:, :])
                    else:
                        nc.vector.tensor_tensor(Rt[:spn, :], Rt[:spn, :],
                                                E2[:spn, :], OP.add)
                    Et[j] = E2
                    pend.append(j)
                    # lag 12 early (lets the deferred finalize borrow the
                    # AV psum tags), tapering to 4 so the end-of-slot AV
                    # flush stays small (no ACT bubble)
                    if len(pend) > 11:
                        emit_av(pend.pop(0))
                    if idx >= 18 and len(pend) > 3:
                        emit_av(pend.pop(0))
                for j in pend:
                    emit_av(j)
                pend = []
                if finalize_prev:
                    for piece in finalize_prev:
                        piece()
                nc.vector.tensor_copy(out=Lsb[:], in_=pacc.pop('l')[:])
                nc.vector.tensor_copy(out=Msb[:], in_=pacc.pop('m')[:])
                finalize_prev = make_finalize(k, Rt, Lsb, Msb)
                finalize_prev.pop(0)()      # den+recip right at slot end
            for piece in finalize_prev:
                piece()
    nc.compile()
    return nc


def make_in_maps(x, forward_memory, reverse_memory, ctrl, Wq, Wk, Wv, Wo,
                 Wc, Wg, bg):
    f = np.float32
    import ml_dtypes
    bf = ml_dtypes.bfloat16
    # causal bias constants: bias[i,c] = -M * #{k: c-sh < k <= i}
    # = tri_neg.T @ band, band[k,u] = (u < k + 1920 - 1024*half)
    MNEG = 1.0e4
    kk = np.arange(P).reshape(P, 1)
    ii = np.arange(P).reshape(1, P)
    trineg = (-MNEG * (ii >= kk)).astype(np.float16)       # lhsT [k, i]
    uu = np.arange(3072).reshape(1, 3072)
    # host-side tiny precomputes (fused-query gate + ctrl bias)
    cq = (Wc @ ctrl).astype(f)                   # [C] = Wc @ ctrl
    wf_full = (Wg @ Wq).astype(f)                # [H, C] rows = Wg_h @ Wq
    gb_full = (Wg @ cq + bg).astype(f)           # [H]
    in_maps = []
    for core in range(8):
        b, g = core // 4, core % 4
        units = slot_units(g)
        hp, hs, _ = GROUP_MAP[g]
        kv = np.concatenate(
            [x[b], forward_memory[b], reverse_memory[b]], axis=0)
        kvT = np.ascontiguousarray(kv.T, dtype=f)
        xqT = np.concatenate(
            [np.ascontiguousarray(x[b, h2 * THALF:(h2 + 1) * THALF, :].T)
             for (_, h2) in units], axis=1)
        wqT = np.concatenate(
            [np.ascontiguousarray(Wq[h * P:(h + 1) * P, :].T)
             for (h, _) in units], axis=1)
        wkT0 = np.ascontiguousarray(Wk[hp * P:(hp + 1) * P, :].T)
        wkT1 = np.ascontiguousarray(Wk[hs * P:(hs + 1) * P, :].T)
        wvT2 = np.concatenate(
            [np.ascontiguousarray(Wv[h * P:(h + 1) * P, :].T)
             for h in (hp, hs)], axis=1)
        woT = np.concatenate(
            [np.ascontiguousarray(Wo[:, h * P:(h + 1) * P].T)
             for (h, _) in units], axis=1)
        wfT = np.stack([wf_full[h, :] for (h, _) in units], axis=1)  # [C,3]
        qbs = np.stack([cq[h * P:(h + 1) * P] for (h, _) in units],
                       axis=1)                                       # [P,3]
        gb3 = np.array([[gb_full[h] for (h, _) in units]], dtype=f)
        half2 = units[2][1]
        band01 = (uu < kk + 1920).astype(np.float16)
        band2 = (uu < kk + 1920 - THALF * half2).astype(np.float16)
        bandm = np.concatenate([band01, band2], axis=1)      # [P, 2*3072]
        in_maps.append({
            "kvT": kvT.astype(bf), "xqT": np.ascontiguousarray(
                xqT, dtype=f).astype(bf),
            "wqT": np.ascontiguousarray(wqT, dtype=f).astype(bf),
            "wkT0": wkT0.astype(bf), "wkT1": wkT1.astype(bf),
            "wvT2": np.ascontiguousarray(wvT2, dtype=f).astype(bf),
            "woT": np.ascontiguousarray(woT, dtype=f),
            "wfT": np.ascontiguousarray(wfT, dtype=f).astype(bf),
            "qbs": np.ascontiguousarray(qbs, dtype=f),
            "gb3": gb3,
            "bandm": bandm, "trineg": trineg,
            "ones_r": np.ones((1, P), dtype=f),
        })
    return in_maps


def unshard(results):
    y = np.zeros((B, T, C), dtype=np.float32)
    for core in range(8):
        b, g = core // 4, core % 4
        ypc = results[core]["yp"].astype(np.float32)
        for kslot, (_, half) in enumerate(slot_units(g)):
            y[b, half * THALF:(half + 1) * THALF, :] += \
                ypc[kslot * C:(kslot + 1) * C, :].T
    return y


_nc_cache = {}


def _get_nc(use_f32r=True, debug=False, att_bf16=True):
    key = (use_f32r, debug, att_bf16)
    if key not in _nc_cache:
        _nc_cache[key] = build_nc(use_f32r, debug, att_bf16)
    return _nc_cache[key]


def kernel(**inputs):
    return kernel_ex(**inputs)[0]


def kernel_ex(trace=False, trace_cores=None, use_f32r=True, debug=False,
              att_bf16=True, **inputs):
    from concourse.bass_utils import run_bass_kernel_spmd

    np_inputs = {k: np.asarray(v) for k, v in inputs.items()}
    in_maps = make_in_maps(**np_inputs)
    nc = _get_nc(use_f32r, debug, att_bf16)
    res = run_bass_kernel_spmd(nc, in_maps, list(range(8)), trace=trace,
                               trace_cores=trace_cores)
    return unshard(res.results), res


# revision 39
# speedup vs baseline: 1.1567x; 1.1567x over previous
"""Trainium2 Bass kernel for nn_CMAModel (control-fused memory attention).

Math (reference):
  q  = x @ Wq.T + ctrl @ Wc.T                  [B,T,C]
  kv = [x; fwd_mem; rev_mem]                   [B,S,C], S = T+M+R = 5440
  k  = kv @ Wk.T ; v = kv @ Wv.T
  per head h (D=128): scores = q_h k_h^T / sqrt(D), causal mask on the
  local T block only; w = softmax(scores); out_h = w_loc v_loc + gate_h *
  (w_mem v_mem); gate = sigmoid(q @ Wg.T + bg); y = concat(out_h) @ Wo.T

Sharding (8 cores, SPMD — one program, per-core behavior via input data):
  core = b*4 + g  (b = batch, g = group 0..3).  24 units of (b, head,
  T-half).  Each core runs 3 "slots": slots 0,1 = both halves of a
  "pair" head, slot 2 = one half of a "single" head (shared with the
  neighbor core).  Per batch:
    g=0: pair h0, single (h1, half A)     g=1: pair h2, single (h1, B)
    g=2: pair h3, single (h4, half A)     g=3: pair h5, single (h4, B)
  K/V are computed on-device per head-cache (cache0 = pair head,
  cache1 = single head) from the core's batch kv, column-sliced weights.

Layouts: everything feature-major ([C, tokens]) so all matmuls are
  natural (lhsT = transposed weights supplied by the host; no on-device
  transposes).  Attention uses scoresT [s, t]: softmax denominators are
  per-t sums over the s (partition) axis, accumulated per-partition into
  a running R on DVE; the final cross-partition sum AND its broadcast to
  all 128 partitions happen in ONE matmul (all-ones [128,128] @ R).
  Causal masking is (iota >= thr) with host-supplied per-partition
  thresholds — fully data-driven, identical control flow on all cores.

Fused-query gate / ctrl bias are tiny host-side precomputes:
  wfT = (Wg_h @ Wq) per slot, qbs = (Wc @ ctrl) slice, gb3 = gate bias.

Output: per-slot out-projection partials y_p = Wo[:, h-slice].T-free
  contribution [768, 1024] in bf16; the host sums the 6 head partials
  per (batch, half) and transposes — the standard row-parallel unshard.
"""

import numpy as np

B, T, C, H, M, R = 2, 2048, 768, 6, 3072, 320
D = C // H          # 128
S = T + M + R       # 5440
P = 128
NT = (S + P - 1) // P          # 43 s-tiles (last has 64 rows)
NLOC = T // P                  # 16 local s-tiles
NCT = C // P                   # 6 feature tiles
THALF = T // 2                 # 1024
NCH = THALF // 512             # 2 chunks of 512 per half
DSCALE = float(D) ** -0.5

# per-batch slot maps: (pair_head, single_head, single_half) per group
GROUP_MAP = [(0, 1, 0), (2, 1, 1), (3, 4, 0), (5, 4, 1)]


def slot_units(g):
    hp, hs, hsh = GROUP_MAP[g]
    return [(hp, 0), (hp, 1), (hs, hsh)]


def _kchunks():
    out = []
    off = 0
    while off < S:
        w = min(512, S - off)
        out.append((off, w))
        off += w
    return out


KCH = _kchunks()               # 10x512 + 320


def build_nc(use_f32r=True, debug=False, att_bf16=True):
    import concourse.mybir as mybir
    import concourse.tile as tile
    from concourse import bacc

    f32 = mybir.dt.float32
    f32r = mybir.dt.float32r if use_f32r else f32
    adt = mybir.dt.bfloat16 if att_bf16 else f32r
    AF = mybir.ActivationFunctionType
    OP = mybir.AluOpType

    def mm(psum, lhsT, rhs, start=True, stop=True):
        nc.tensor.matmul(psum, lhsT, rhs, start=start, stop=stop)

    nc = bacc.Bacc("TRN2", target_bir_lowering=False, debug=False,
                   num_devices=8)

    dram = {}
    for name, shape in [
        ("kvT", [C, S]),            # batch kv, transposed
        ("xqT", [C, 3 * THALF]),    # per-slot x columns, transposed
        ("wqT", [C, 3 * P]),        # per-slot Wq head-rows, transposed
        ("wkT0", [C, P]),           # pair-head Wk rows, transposed
        ("wkT1", [C, P]),           # single-head Wk rows, transposed
        ("wvT2", [C, 2 * P]),       # [pair | single] Wv rows, transposed
        ("woT", [P, 3 * C]),        # per-slot Wo head-cols, transposed
        ("wfT", [C, 3]),            # per-slot fused gate weights (Wg_h@Wq)
        ("qbs", [P, 3]),            # per-slot q bias column (Wc@ctrl slice)
        ("gb3", [1, 3]),            # per-slot full gate bias
        ("ones_r", [1, P]),         # ones row (f32 bcast stationary)
        ("bandm", [P, 2 * 3072]),   # fp16 causal bias bands (slots01|slot2)
        ("trineg", [P, P]),         # fp16 -M lower-tri (bias stationary)
    ]:
        dt_ = f32
        if name in ("kvT", "xqT", "wqT", "wkT0", "wkT1", "wvT2", "wfT"):
            dt_ = mybir.dt.bfloat16
        if name == "woT":
            dt_ = f32 if att_bf16 else f32r
        if name in ("bandm", "trineg"):
            dt_ = mybir.dt.float16
        dram[name] = nc.dram_tensor(name, shape, dt_, kind="ExternalInput")
    yp = nc.dram_tensor("yp", [3 * C, THALF], mybir.dt.bfloat16,
                        kind="ExternalOutput")
    dbg = {}
    if debug:
        for name, shape in [("d_q", [P, 3 * THALF]), ("d_gate", [1, 3 * THALF]),
                            ("d_kh0", [P, 1024]), ("d_vh", [P, 512]),
                            ("d_att", [P, 3 * THALF])]:
            dbg[name] = nc.dram_tensor(name, shape, f32,
                                       kind="ExternalOutput")

    from contextlib import ExitStack

    with tile.TileContext(nc) as tc, ExitStack() as _ctx:
        consts = _ctx.enter_context(tc.tile_pool(name="consts", bufs=1))
        # ---- K/V weights first (single rearranged DMAs — keep the
        # gpsimd queue short so the kv stream starts immediately) ----
        wk0 = consts.tile([P, NCT, P], adt)
        wk1 = consts.tile([P, NCT, P], adt)
        wv2 = consts.tile([P, NCT, 2 * P], adt)
        nc.gpsimd.dma_start(out=wk0[:], in_=dram["wkT0"][:, :].rearrange(
            "(a p) d -> p a d", p=P))
        nc.gpsimd.dma_start(out=wk1[:], in_=dram["wkT1"][:, :].rearrange(
            "(a p) d -> p a d", p=P))
        nc.gpsimd.dma_start(out=wv2[:], in_=dram["wvT2"][:, :].rearrange(
            "(a p) d -> p a d", p=P))
        ones_row = consts.tile([1, P], f32)
        nc.sync.dma_start(out=ones_row[:], in_=dram["ones_r"][:, :])
        ones_sq = consts.tile([P, P], mybir.dt.float16)
        nc.vector.memset(ones_sq[:], 1.0)

        # ---- phase 2: K/V projections into SBUF caches; q inputs and
        # weights stream in alongside (interleaved, both queues) ----
        f16 = mybir.dt.float16
        kh0 = consts.tile([P, S], adt)
        kh1 = consts.tile([P, S], adt)
        vh = consts.tile([P, NT, 2 * P], f16)
        xq_all = consts.tile([P, NCT, 3 * THALF], adt)
        xq_parts = [(ct, c0) for ct in range(NCT)
                    for c0 in range(0, 3 * THALF, 512)]  # 36 x [128,512]
        with tc.tile_pool(name="kvp", bufs=8) as kvp, \
             tc.tile_pool(name="kvps", bufs=1, space="PSUM") as kvps:
            for sc, (off, w) in enumerate(KCH):
                pk0 = kvps.tile([P, 512], f32, tag="k0", bufs=2)
                pk1 = kvps.tile([P, 512], f32, tag="k1", bufs=2)
                subs = []
                o2 = off
                while o2 < off + w:
                    subs.append((o2 - off, min(P, off + w - o2)))
                    o2 += P
                pv = [kvps.tile([P, 2 * P], f32, tag=f"v{si}",
                                name=f"pv{si}", bufs=1)
                      for si in range(len(subs))]
                j0 = off // P
                for ct in range(NCT):
                    kv_t = kvp.tile([P, 512], adt, tag="kv")
                    eng = nc.sync if (sc * NCT + ct) % 2 == 0 else nc.gpsimd
                    eng.dma_start(
                        out=kv_t[:, :w],
                        in_=dram["kvT"][ct * P:(ct + 1) * P, off:off + w])
                    mm(pk0[:, :w], wk0[:, ct, :], kv_t[:, :w],
                       start=(ct == 0), stop=(ct == NCT - 1))
                    mm(pk1[:, :w], wk1[:, ct, :], kv_t[:, :w],
                       start=(ct == 0), stop=(ct == NCT - 1))
                    for si, (so, sw) in enumerate(subs):
                        mm(pv[si][:sw, :], kv_t[:, so:so + sw],
                           wv2[:, ct, :],
                           start=(ct == 0), stop=(ct == NCT - 1))
                        if ct == NCT - 1:
                            # drain each V sub right after its stop so
                            # the next chunk's V matmuls aren't blocked
                            nc.vector.tensor_copy(out=vh[:sw, j0 + si, :],
                                                  in_=pv[si][:sw, :])
                nc.vector.tensor_copy(out=kh0[:, off:off + w],
                                      in_=pk0[:, :w])
                nc.vector.tensor_copy(out=kh1[:, off:off + w],
                                      in_=pk1[:, :w])
                # x columns for q-proj: 4 pieces per chunk from sc=2,
                # alternating queues, so neither engine backs up
                if sc >= 2:
                    for pi in range(4):
                        k4 = (sc - 2) * 4 + pi
                        if k4 < len(xq_parts):
                            ct, c0 = xq_parts[k4]
                            eng = nc.sync if k4 % 2 == 0 else nc.gpsimd
                            eng.dma_start(
                                out=xq_all[:, ct, c0:c0 + 512],
                                in_=dram["xqT"][ct * P:(ct + 1) * P,
                                                c0:c0 + 512])

        # ---- remaining constants (after the kv stream is queued) ----
        wqt = consts.tile([P, NCT, 3 * P], adt)
        wfT3 = consts.tile([P, NCT, 3], adt)
        nc.gpsimd.dma_start(out=wqt[:], in_=dram["wqT"][:, :].rearrange(
            "(a p) d -> p a d", p=P))
        nc.gpsimd.dma_start(out=wfT3[:], in_=dram["wfT"][:, :].rearrange(
            "(a p) d -> p a d", p=P))
        qbs = consts.tile([P, 3], f32)
        nc.gpsimd.dma_start(out=qbs[:], in_=dram["qbs"][:, :])
        gb3 = consts.tile([1, 3], f32)
        nc.gpsimd.dma_start(out=gb3[:], in_=dram["gb3"][:, :])
        wot = consts.tile([P, 3 * C], adt)
        if att_bf16:
            nc.gpsimd.dma_start(out=wot[:], in_=dram["woT"][:, :])
        else:
            nc.sync.dma_start(out=wot[:], in_=dram["woT"][:, :])
        bandm = consts.tile([P, 2, 3072], f16)
        nc.gpsimd.dma_start(out=bandm[:],
                            in_=dram["bandm"][:, :].rearrange(
                                "p (a b) -> p a b", a=2))
        trineg = consts.tile([P, P], f16)
        nc.gpsimd.dma_start(out=trineg[:], in_=dram["trineg"][:, :])

        # ---- phase 3: q projection + gate (all inputs SBUF-resident) ----
        qsb = consts.tile([P, 3, THALF], adt)
        gate = consts.tile([1, 3, THALF], f32)
        with tc.tile_pool(name="qps", bufs=1, space="PSUM") as qps:
            for k in range(3):
                for ch in range(NCH):
                    pq = qps.tile([P, 512], f32, tag="q", bufs=3)
                    pg = qps.tile([1, 512], f32, tag="g", bufs=3)
                    for ct in range(NCT):
                        xs = xq_all[:, ct, k * THALF + ch * 512:
                                    k * THALF + (ch + 1) * 512]
                        mm(pq[:], wqt[:, ct, k * P:(k + 1) * P], xs,
                           start=(ct == 0), stop=(ct == NCT - 1))
                        mm(pg[:], wfT3[:, ct, k:k + 1], xs,
                           start=(ct == 0), stop=(ct == NCT - 1))
                    nc.vector.tensor_scalar_add(
                        qsb[:, k, ch * 512:(ch + 1) * 512], pq[:],
                        qbs[:, k:k + 1])
                    nc.scalar.activation(
                        gate[0:1, k, ch * 512:(ch + 1) * 512], pg[:],
                        AF.Sigmoid, bias=gb3[0:1, k:k + 1], scale=1.0)

        if debug:
            nc.gpsimd.dma_start(out=dbg["d_q"][:, :],
                               in_=qsb[:].rearrange("p a b -> p (a b)"))
            nc.sync.dma_start(out=dbg["d_gate"][0:1, :],
                              in_=gate[:].rearrange("p a b -> p (a b)"))
            nc.gpsimd.dma_start(out=dbg["d_kh0"][:, :], in_=kh0[:, 0:1024])
            nc.gpsimd.dma_start(out=dbg["d_vh"][:, :],
                               in_=vh[:, 0:2, :].rearrange("p a b -> p (a b)"))
        # ---- phase 4: attention + output projection, per slot ----
        # Per slot: one s-tile does 2 N=512 scoresT matmuls into one
        # 2-bank psum tile, one 1024-wide exp, a causal mask multiply on
        # diagonal tiles (precomputed mask tiles for slots 0/1 whose
        # halves are compile-time; threshold-generated for slot 2), a
        # single fp16 running-sum add (softmax denominator), and two AV
        # accumulations.  The slot-end normalization/out-projection chain
        # is DEFERRED into the next slot's j-loop so the PE never idles
        # across slot boundaries (keeps HAM warm).
        with tc.tile_pool(name="att", bufs=2) as att_pool, \
             tc.tile_pool(name="ep", bufs=15) as ep, \
             tc.tile_pool(name="vec", bufs=3) as vec, \
             tc.tile_pool(name="cmb", bufs=1) as cmb, \
             tc.tile_pool(name="ysb", bufs=3) as ysb, \
             tc.tile_pool(name="aps", bufs=1, space="PSUM") as aps:
            finalize_prev = None

            def make_finalize(k, Rt, Lsb, Msb):
                # psum comes from the AV-region tags ("mm"/"ll") which
                # are free early in the next slot's loop — NEVER from
                # the "sc" tag, whose 2-buf rotation would stall the
                # next slot's scores->exp pipeline behind this chain.
                # Emission is SPLIT into pieces consumed one per j-loop
                # iteration so the dependent out-proj matmuls land in
                # the PE stream after the DVE chain has had time to run.
                state = {}

                def fin_early():
                    # denominator: sum over partitions AND broadcast to
                    # all 128 rows in one all-ones matmul; ONE wide
                    # reciprocal (DVE recip has a ~3.3us fixed cost).
                    # Runs at the END of slot k while DVE is quiet —
                    # ready long before the out-proj needs it.
                    prb = aps.tile([P, NCH, 512], f32, tag="mm",
                                   name="prb")
                    for ch in range(NCH):
                        mm(prb[:, ch, :], ones_sq[:],
                           Rt[:, ch * 512:(ch + 1) * 512])
                    rb = cmb.tile([P, NCH, 512], f32r, tag="rb")
                    with nc.allow_low_precision(reason="f32r norm"):
                        nc.vector.reciprocal(
                            rb[:].rearrange("p a b -> p (a b)"),
                            prb[:].rearrange("p a b -> p (a b)"))
                    state['rb'] = rb

                def fin_head():
                    # UNNORMALIZED attb' = Lsb + gate*Msb (bf16); the
                    # 1/den scale is folded into the yt output multiply
                    attb = att_pool.tile([P, NCH, 512], adt, tag="attb")
                    pgb = aps.tile([P, NCH, 512], f32, tag="mm",
                                   name="pgb")
                    for ch in range(NCH):
                        mm(pgb[:, ch, :], ones_row[:],
                           gate[0:1, k, ch * 512:(ch + 1) * 512])
                    for ch in range(NCH):
                        t2 = cmb.tile([P, 512], f32, tag="t2")
                        nc.vector.tensor_tensor(t2[:], Msb[:, ch, :],
                                                pgb[:, ch, :], OP.mult)
                        with nc.allow_low_precision(reason="bf16 att"):
                            nc.vector.tensor_tensor(attb[:, ch, :], t2[:],
                                                    Lsb[:, ch, :], OP.add)
                    state['attb'] = attb

                def fin_py(ot):
                    py = aps.tile([P, NCH, 512], f32, tag="ll",
                                  name=f"py{ot}")
                    for ch in range(NCH):
                        mm(py[:, ch, :],
                           wot[:, k * C + ot * P:k * C + (ot + 1) * P],
                           state['attb'][:, ch, :])
                    yt = ysb.tile([P, NCH, 512], mybir.dt.bfloat16,
                                  tag="y")
                    with nc.allow_low_precision(reason="bf16 out"):
                        nc.vector.tensor_tensor(
                            yt[:].rearrange("p a b -> p (a b)"),
                            py[:].rearrange("p a b -> p (a b)"),
                            state['rb'][:].rearrange("p a b -> p (a b)"),
                            OP.mult)
                    nc.sync.dma_start(
                        out=yp[k * C + ot * P:k * C + (ot + 1) * P, :],
                        in_=yt[:].rearrange("p a b -> p (a b)"))

                return [fin_early, fin_head] + \
                    [lambda ot=ot: fin_py(ot) for ot in range(NCT)]

            for k in range(3):
                kh = kh0 if k < 2 else kh1
                voff = 0 if k < 2 else P
                loc_end = 8 if k == 0 else NLOC
                msk_lo = {0: 0, 1: 8, 2: 0}[k]
                # locals first, then memory: the "mm" (memory-AV) psum
                # banks stay free until idx ~16+lag, giving the deferred
                # finalize of the previous slot its psum without ever
                # touching the scores rotation
                js = list(range(loc_end)) + list(range(NLOC, NT))
                Rt = vec.tile([P, THALF], mybir.dt.float16, tag="R")
                Lsb = att_pool.tile([P, NCH, 512], f32, tag="Lsb")
                Msb = att_pool.tile([P, NCH, 512], f32, tag="Msb")
                qrhs = qsb[:, k, :]
                pacc = {}
                Et = {}
                pend = []

                def emit_av(j, k=k, voff=voff, loc_end=loc_end, pacc=pacc,
                            Et=Et):
                    spn = min(P, S - j * P)
                    E2 = Et.pop(j)
                    reg = 'l' if j < NLOC else 'm'
                    first = j == 0 or j == NLOC
                    last = j == loc_end - 1 or j == NT - 1
                    if first:
                        pacc[reg] = aps.tile(
                            [P, NCH, 512], f32,
                            tag=("ll" if reg == 'l' else "mm"),
                            name=f"pacc{reg}")
                    for ch in range(NCH):
                        mm(pacc[reg][:, ch, :], vh[:spn, j, voff:voff + P],
                           E2[:spn, ch * 512:(ch + 1) * 512],
                           start=first, stop=last)

                for idx, j in enumerate(js):
                    # one finalize piece every other iteration from idx 8:
                    # head (den/gate/attb chain) at 8, out-proj pieces at
                    # 10,12,... so each py matmul finds attb/yt ready
                    if idx >= 8 and idx % 2 == 0 and finalize_prev:
                        finalize_prev.pop(0)()
                    spn = min(P, S - j * P)
                    masked = msk_lo <= j < loc_end
                    ps = aps.tile([P, NCH, 512], f32, tag="sc", bufs=2)
                    for ch in range(NCH):
                        mm(ps[:spn, ch, :], kh[:, j * P:j * P + spn],
                           qrhs[:, ch * 512:(ch + 1) * 512],
                           start=True, stop=not masked)
                    if masked:
                        # additive causal bias: -M * count(t < s) via
                        # (-M*lower_tri) @ shifted 0/1 band; per-core
                        # half-offset is baked into the band data
                        bsl = 1 if k == 2 else 0
                        u0 = 1920 - 128 * (j - msk_lo)
                        for ch in range(NCH):
                            mm(ps[:spn, ch, :], trineg[:, :spn],
                               bandm[:, bsl, u0 + ch * 512:
                                     u0 + (ch + 1) * 512],
                               start=False, stop=True)
                    E2 = ep.tile([P, THALF], f16, tag="E")
                    nc.scalar.activation(E2[:spn], ps[:spn].rearrange(
                        "p a b -> p (a b)"), AF.Exp, scale=DSCALE)
                    if idx == 0:
                        nc.vector.tensor_copy(out=Rt[:, :], in_=E2[:, :])
                    else:
                        nc.vector.tensor_tensor(Rt[:spn, :], Rt[:spn, :],
                                                E2[:spn, :], OP.add)
                    Et[j] = E2
                    pend.append(j)
                    # lag 12 early (lets the deferred finalize borrow the
                    # AV psum tags), tapering to 4 so the end-of-slot AV
                    # flush stays small (no ACT bubble)
                    if len(pend) > 11:
                        emit_av(pend.pop(0))
                    if idx >= 26 and len(pend) > 4:
                        emit_av(pend.pop(0))
                for j in pend:
                    emit_av(j)
                pend = []
                if finalize_prev:
                    for piece in finalize_prev:
                        piece()
                nc.vector.tensor_copy(out=Lsb[:], in_=pacc.pop('l')[:])
                nc.vector.tensor_copy(out=Msb[:], in_=pacc.pop('m')[:])
                finalize_prev = make_finalize(k, Rt, Lsb, Msb)
                finalize_prev.pop(0)()      # den+recip right at slot end
            for piece in finalize_prev:
                piece()
    nc.compile()
    return nc


def make_in_maps(x, forward_memory, reverse_memory, ctrl, Wq, Wk, Wv, Wo,
                 Wc, Wg, bg):
    f = np.float32
    import ml_dtypes
    bf = ml_dtypes.bfloat16
    # causal bias constants: bias[i,c] = -M * #{k: c-sh < k <= i}
    # = tri_neg.T @ band, band[k,u] = (u < k + 1920 - 1024*half)
    MNEG = 1.0e4
    kk = np.arange(P).reshape(P, 1)
    ii = np.arange(P).reshape(1, P)
    trineg = (-MNEG * (ii >= kk)).astype(np.float16)       # lhsT [k, i]
    uu = np.arange(3072).reshape(1, 3072)
    # host-side tiny precomputes (fused-query gate + ctrl bias)
    cq = (Wc @ ctrl).astype(f)                   # [C] = Wc @ ctrl
    wf_full = (Wg @ Wq).astype(f)                # [H, C] rows = Wg_h @ Wq
    gb_full = (Wg @ cq + bg).astype(f)           # [H]
    in_maps = []
    for core in range(8):
        b, g = core // 4, core % 4
        units = slot_units(g)
        hp, hs, _ = GROUP_MAP[g]
        kv = np.concatenate(
            [x[b], forward_memory[b], reverse_memory[b]], axis=0)
        kvT = np.ascontiguousarray(kv.T, dtype=f)
        xqT = np.concatenate(
            [np.ascontiguousarray(x[b, h2 * THALF:(h2 + 1) * THALF, :].T)
             for (_, h2) in units], axis=1)
        wqT = np.concatenate(
            [np.ascontiguousarray(Wq[h * P:(h + 1) * P, :].T)
             for (h, _) in units], axis=1)
        wkT0 = np.ascontiguousarray(Wk[hp * P:(hp + 1) * P, :].T)
        wkT1 = np.ascontiguousarray(Wk[hs * P:(hs + 1) * P, :].T)
        wvT2 = np.concatenate(
            [np.ascontiguousarray(Wv[h * P:(h + 1) * P, :].T)
             for h in (hp, hs)], axis=1)
        woT = np.concatenate(
            [np.ascontiguousarray(Wo[:, h * P:(h + 1) * P].T)
             for (h, _) in units], axis=1)
        wfT = np.stack([wf_full[h, :] for (h, _) in units], axis=1)  # [C,3]
        qbs = np.stack([cq[h * P:(h + 1) * P] for (h, _) in units],
                       axis=1)                                       # [P,3]
        gb3 = np.array([[gb_full[h] for (h, _) in units]], dtype=f)
        half2 = units[2][1]
        band01 = (uu < kk + 1920).astype(np.float16)
        band2 = (uu < kk + 1920 - THALF * half2).astype(np.float16)
        bandm = np.concatenate([band01, band2], axis=1)      # [P, 2*3072]
        in_maps.append({
            "kvT": kvT.astype(bf), "xqT": np.ascontiguousarray(
                xqT, dtype=f).astype(bf),
            "wqT": np.ascontiguousarray(wqT, dtype=f).astype(bf),
            "wkT0": wkT0.astype(bf), "wkT1": wkT1.astype(bf),
            "wvT2": np.ascontiguousarray(wvT2, dtype=f).astype(bf),
            "woT": np.ascontiguousarray(woT, dtype=f),
            "wfT": np.ascontiguousarray(wfT, dtype=f).astype(bf),
            "qbs": np.ascontiguousarray(qbs, dtype=f),
            "gb3": gb3,
            "bandm": bandm, "trineg": trineg,
            "ones_r": np.ones((1, P), dtype=f),
        })
    return in_maps


def unshard(results):
    y = np.zeros((B, T, C), dtype=np.float32)
    for core in range(8):
        b, g = core // 4, core % 4
        ypc = results[core]["yp"].astype(np.float32)
        for kslot, (_, half) in enumerate(slot_units(g)):
            y[b, half * THALF:(half + 1) * THALF, :] += \
                ypc[kslot * C:(kslot + 1) * C, :].T
    return y


_nc_cache = {}


def _get_nc(use_f32r=True, debug=False, att_bf16=True):
    key = (use_f32r, debug, att_bf16)
    if key not in _nc_cache:
        _nc_cache[key] = build_nc(use_f32r, debug, att_bf16)
    return _nc_cache[key]


def kernel(**inputs):
    return kernel_ex(**inputs)[0]


def kernel_ex(trace=False, trace_cores=None, use_f32r=True, debug=False,
              att_bf16=True, **inputs):
    from concourse.bass_utils import run_bass_kernel_spmd

    np_inputs = {k: np.asarray(v) for k, v in inputs.items()}
    in_maps = make_in_maps(**np_inputs)
    nc = _get_nc(use_f32r, debug, att_bf16)
    res = run_bass_kernel_spmd(nc, in_maps, list(range(8)), trace=trace,
                               trace_cores=trace_cores)
    return unshard(res.results), res


# revision 48
# speedup vs baseline: 1.1951x; 1.0332x over previous
"""Trainium2 Bass kernel for nn_CMAModel (control-fused memory attention).

Math (reference):
  q  = x @ Wq.T + ctrl @ Wc.T                  [B,T,C]
  kv = [x; fwd_mem; rev_mem]                   [B,S,C], S = T+M+R = 5440
  k  = kv @ Wk.T ; v = kv @ Wv.T
  per head h (D=128): scores = q_h k_h^T / sqrt(D), causal mask on the
  local T block only; w = softmax(scores); out_h = w_loc v_loc + gate_h *
  (w_mem v_mem); gate = sigmoid(q @ Wg.T + bg); y = concat(out_h) @ Wo.T

Sharding (8 cores, SPMD — one program, per-core behavior via input data):
  core = b*4 + g  (b = batch, g = group 0..3).  24 units of (b, head,
  T-half).  Each core runs 3 "slots": slots 0,1 = both halves of a
  "pair" head, slot 2 = one half of a "single" head (shared with the
  neighbor core).  Per batch:
    g=0: pair h0, single (h1, half A)     g=1: pair h2, single (h1, B)
    g=2: pair h3, single (h4, half A)     g=3: pair h5, single (h4, B)
  K/V are computed on-device per head-cache (cache0 = pair head,
  cache1 = single head) from the core's batch kv, column-sliced weights.

Layouts: everything feature-major ([C, tokens]) so all matmuls are
  natural (lhsT = transposed weights supplied by the host; no on-device
  transposes).  Attention uses scoresT [s, t]: softmax denominators are
  per-t sums over the s (partition) axis, accumulated per-partition into
  a running R on DVE; the final cross-partition sum AND its broadcast to
  all 128 partitions happen in ONE matmul (all-ones [128,128] @ R).
  Causal masking is (iota >= thr) with host-supplied per-partition
  thresholds — fully data-driven, identical control flow on all cores.

Fused-query gate / ctrl bias are tiny host-side precomputes:
  wfT = (Wg_h @ Wq) per slot, qbs = (Wc @ ctrl) slice, gb3 = gate bias.

Output: per-slot out-projection partials y_p = Wo[:, h-slice].T-free
  contribution [768, 1024] in bf16; the host sums the 6 head partials
  per (batch, half) and transposes — the standard row-parallel unshard.
"""

import numpy as np

B, T, C, H, M, R = 2, 2048, 768, 6, 3072, 320
D = C // H          # 128
S = T + M + R       # 5440
P = 128
NT = (S + P - 1) // P          # 43 s-tiles (last has 64 rows)
NLOC = T // P                  # 16 local s-tiles
NCT = C // P                   # 6 feature tiles
THALF = T // 2                 # 1024
NCH = THALF // 512             # 2 chunks of 512 per half
DSCALE = float(D) ** -0.5

# per-batch slot maps: (pair_head, single_head, single_half) per group
GROUP_MAP = [(0, 1, 0), (2, 1, 1), (3, 4, 0), (5, 4, 1)]


def slot_units(g):
    hp, hs, hsh = GROUP_MAP[g]
    return [(hp, 0), (hp, 1), (hs, hsh)]


def _kchunks():
    out = []
    off = 0
    while off < S:
        w = min(512, S - off)
        out.append((off, w))
        off += w
    return out


KCH = _kchunks()               # 10x512 + 320


def build_nc(use_f32r=True, debug=False, att_bf16=True):
    import concourse.mybir as mybir
    import concourse.tile as tile
    from concourse import bacc

    f32 = mybir.dt.float32
    f32r = mybir.dt.float32r if use_f32r else f32
    adt = mybir.dt.bfloat16 if att_bf16 else f32r
    AF = mybir.ActivationFunctionType
    OP = mybir.AluOpType

    def mm(psum, lhsT, rhs, start=True, stop=True):
        nc.tensor.matmul(psum, lhsT, rhs, start=start, stop=stop)

    nc = bacc.Bacc("TRN2", target_bir_lowering=False, debug=False,
                   num_devices=8)

    dram = {}
    for name, shape in [
        ("kvT", [C, S]),            # batch kv, transposed
        ("xqT", [C, 3 * THALF]),    # per-slot x columns, transposed
        ("wqT", [C, 3 * P]),        # per-slot Wq head-rows, transposed
        ("wkT0", [C, P]),           # pair-head Wk rows, transposed
        ("wkT1", [C, P]),           # single-head Wk rows, transposed
        ("wvT2", [C, 2 * P]),       # [pair | single] Wv rows, transposed
        ("woT", [P, 3 * C]),        # per-slot Wo head-cols, transposed
        ("qbs", [P, 3]),            # per-slot q bias column (Wc@ctrl slice)
        ("gateh", [1, 3 * THALF]),  # per-slot gate rows (host sigmoid)
        ("ones_r", [1, P]),         # ones row (f32 bcast stationary)
        ("bandm", [P, 2 * 3072]),   # fp16 causal bias bands (slots01|slot2)
        ("trineg", [P, P]),         # fp16 -M lower-tri (bias stationary)
    ]:
        dt_ = f32
        if name in ("kvT", "xqT", "wqT", "wkT0", "wkT1", "wvT2"):
            dt_ = mybir.dt.bfloat16
        if name == "woT":
            dt_ = f32 if att_bf16 else f32r
        if name in ("bandm", "trineg"):
            dt_ = mybir.dt.float16
        dram[name] = nc.dram_tensor(name, shape, dt_, kind="ExternalInput")
    yp = nc.dram_tensor("yp", [3 * C, THALF], mybir.dt.bfloat16,
                        kind="ExternalOutput")
    dbg = {}
    if debug:
        for name, shape in [("d_q", [P, 3 * THALF]), ("d_gate", [1, 3 * THALF]),
                            ("d_kh0", [P, 1024]), ("d_vh", [P, 512]),
                            ("d_att", [P, 3 * THALF])]:
            dbg[name] = nc.dram_tensor(name, shape, f32,
                                       kind="ExternalOutput")

    from contextlib import ExitStack

    with tile.TileContext(nc) as tc, ExitStack() as _ctx:
        consts = _ctx.enter_context(tc.tile_pool(name="consts", bufs=1))
        # ---- K/V weights first (single rearranged DMAs — keep the
        # gpsimd queue short so the kv stream starts immediately) ----
        wk0 = consts.tile([P, NCT, P], adt)
        wk1 = consts.tile([P, NCT, P], adt)
        wv2 = consts.tile([P, NCT, 2 * P], adt)
        nc.gpsimd.dma_start(out=wk0[:], in_=dram["wkT0"][:, :].rearrange(
            "(a p) d -> p a d", p=P))
        nc.gpsimd.dma_start(out=wk1[:], in_=dram["wkT1"][:, :].rearrange(
            "(a p) d -> p a d", p=P))
        nc.gpsimd.dma_start(out=wv2[:], in_=dram["wvT2"][:, :].rearrange(
            "(a p) d -> p a d", p=P))
        ones_row = consts.tile([1, P], f32)
        nc.sync.dma_start(out=ones_row[:], in_=dram["ones_r"][:, :])
        ones_sq = consts.tile([P, P], mybir.dt.float16)
        nc.vector.memset(ones_sq[:], 1.0)

        # ---- phase 2: K/V projections into SBUF caches; q inputs and
        # weights stream in alongside (interleaved, both queues) ----
        f16 = mybir.dt.float16
        kh0 = consts.tile([P, S], adt)
        kh1 = consts.tile([P, S], adt)
        vh = consts.tile([P, NT, 2 * P], f16)
        xq_all = consts.tile([P, NCT, 3 * THALF], adt)
        xq_parts = [(ct, c0) for ct in range(NCT)
                    for c0 in range(0, 3 * THALF, 512)]  # 36 x [128,512]
        with tc.tile_pool(name="kvp", bufs=8) as kvp, \
             tc.tile_pool(name="kvps", bufs=1, space="PSUM") as kvps:
            for sc, (off, w) in enumerate(KCH):
                pk0 = kvps.tile([P, 512], f32, tag="k0", bufs=2)
                pk1 = kvps.tile([P, 512], f32, tag="k1", bufs=2)
                subs = []
                o2 = off
                while o2 < off + w:
                    subs.append((o2 - off, min(P, off + w - o2)))
                    o2 += P
                pv = [kvps.tile([P, 2 * P], f32, tag=f"v{si}",
                                name=f"pv{si}", bufs=1)
                      for si in range(len(subs))]
                j0 = off // P
                for ct in range(NCT):
                    kv_t = kvp.tile([P, 512], adt, tag="kv")
                    eng = nc.sync if (sc * NCT + ct) % 2 == 0 else nc.gpsimd
                    eng.dma_start(
                        out=kv_t[:, :w],
                        in_=dram["kvT"][ct * P:(ct + 1) * P, off:off + w])
                    mm(pk0[:, :w], wk0[:, ct, :], kv_t[:, :w],
                       start=(ct == 0), stop=(ct == NCT - 1))
                    mm(pk1[:, :w], wk1[:, ct, :], kv_t[:, :w],
                       start=(ct == 0), stop=(ct == NCT - 1))
                    for si, (so, sw) in enumerate(subs):
                        mm(pv[si][:sw, :], kv_t[:, so:so + sw],
                           wv2[:, ct, :],
                           start=(ct == 0), stop=(ct == NCT - 1))
                        if ct == NCT - 1:
                            # drain each V sub right after its stop so
                            # the next chunk's V matmuls aren't blocked
                            nc.vector.tensor_copy(out=vh[:sw, j0 + si, :],
                                                  in_=pv[si][:sw, :])
                nc.vector.tensor_copy(out=kh0[:, off:off + w],
                                      in_=pk0[:, :w])
                nc.vector.tensor_copy(out=kh1[:, off:off + w],
                                      in_=pk1[:, :w])
                # x columns for q-proj: 4 pieces per chunk from sc=2,
                # alternating queues, so neither engine backs up
                if sc >= 2:
                    for pi in range(4):
                        k4 = (sc - 2) * 4 + pi
                        if k4 < len(xq_parts):
                            ct, c0 = xq_parts[k4]
                            eng = nc.sync if k4 % 2 == 0 else nc.gpsimd
                            eng.dma_start(
                                out=xq_all[:, ct, c0:c0 + 512],
                                in_=dram["xqT"][ct * P:(ct + 1) * P,
                                                c0:c0 + 512])

        # ---- remaining constants (after the kv stream is queued) ----
        wqt = consts.tile([P, NCT, 3 * P], adt)
        nc.gpsimd.dma_start(out=wqt[:], in_=dram["wqT"][:, :].rearrange(
            "(a p) d -> p a d", p=P))
        qbs = consts.tile([P, 3], f32)
        nc.gpsimd.dma_start(out=qbs[:], in_=dram["qbs"][:, :])
        gate = consts.tile([1, 3, THALF], f32)
        nc.gpsimd.dma_start(out=gate[:], in_=dram["gateh"][:, :].rearrange(
            "p (a b) -> p a b", a=3))
        wot = consts.tile([P, 3 * C], adt)
        if att_bf16:
            nc.gpsimd.dma_start(out=wot[:], in_=dram["woT"][:, :])
        else:
            nc.sync.dma_start(out=wot[:], in_=dram["woT"][:, :])
        bandm = consts.tile([P, 2, 3072], f16)
        nc.gpsimd.dma_start(out=bandm[:],
                            in_=dram["bandm"][:, :].rearrange(
                                "p (a b) -> p a b", a=2))
        trineg = consts.tile([P, P], f16)
        nc.gpsimd.dma_start(out=trineg[:], in_=dram["trineg"][:, :])

        # ---- phase 3: q projection (gate comes precomputed from host) ----
        qsb = consts.tile([P, 3, THALF], adt)
        with tc.tile_pool(name="qps", bufs=1, space="PSUM") as qps:
            for k in range(3):
                for ch in range(NCH):
                    pq = qps.tile([P, 512], f32, tag="q", bufs=3)
                    for ct in range(NCT):
                        xs = xq_all[:, ct, k * THALF + ch * 512:
                                    k * THALF + (ch + 1) * 512]
                        mm(pq[:], wqt[:, ct, k * P:(k + 1) * P], xs,
                           start=(ct == 0), stop=(ct == NCT - 1))
                    nc.vector.tensor_scalar_add(
                        qsb[:, k, ch * 512:(ch + 1) * 512], pq[:],
                        qbs[:, k:k + 1])

        if debug:
            nc.gpsimd.dma_start(out=dbg["d_q"][:, :],
                               in_=qsb[:].rearrange("p a b -> p (a b)"))
            nc.sync.dma_start(out=dbg["d_gate"][0:1, :],
                              in_=gate[:].rearrange("p a b -> p (a b)"))
            nc.gpsimd.dma_start(out=dbg["d_kh0"][:, :], in_=kh0[:, 0:1024])
            nc.gpsimd.dma_start(out=dbg["d_vh"][:, :],
                               in_=vh[:, 0:2, :].rearrange("p a b -> p (a b)"))
        # ---- phase 4: attention + output projection, per slot ----
        # Per slot: one s-tile does 2 N=512 scoresT matmuls into one
        # 2-bank psum tile, one 1024-wide exp, a causal mask multiply on
        # diagonal tiles (precomputed mask tiles for slots 0/1 whose
        # halves are compile-time; threshold-generated for slot 2), a
        # single fp16 running-sum add (softmax denominator), and two AV
        # accumulations.  The slot-end normalization/out-projection chain
        # is DEFERRED into the next slot's j-loop so the PE never idles
        # across slot boundaries (keeps HAM warm).
        with tc.tile_pool(name="att", bufs=2) as att_pool, \
             tc.tile_pool(name="ep", bufs=15) as ep, \
             tc.tile_pool(name="vec", bufs=3) as vec, \
             tc.tile_pool(name="cmb", bufs=1) as cmb, \
             tc.tile_pool(name="ysb", bufs=3) as ysb, \
             tc.tile_pool(name="aps", bufs=1, space="PSUM") as aps:
            finalize_prev = None

            def make_finalize(k, Rt, Lsb, Msb):
                # psum comes from the AV-region tags ("mm"/"ll") which
                # are free early in the next slot's loop — NEVER from
                # the "sc" tag, whose 2-buf rotation would stall the
                # next slot's scores->exp pipeline behind this chain.
                # Emission is SPLIT into pieces consumed one per j-loop
                # iteration so the dependent out-proj matmuls land in
                # the PE stream after the DVE chain has had time to run.
                state = {}

                def fin_early():
                    # denominator: sum over partitions AND broadcast to
                    # all 128 rows in one all-ones matmul; ONE wide
                    # reciprocal (DVE recip has a ~3.3us fixed cost).
                    # Runs at the END of slot k while DVE is quiet —
                    # ready long before the out-proj needs it.
                    prb = aps.tile([P, NCH, 512], f32, tag="mm",
                                   name="prb")
                    for ch in range(NCH):
                        mm(prb[:, ch, :], ones_sq[:],
                           Rt[:, ch * 512:(ch + 1) * 512])
                    rb = cmb.tile([P, NCH, 512], f32r, tag="rb")
                    with nc.allow_low_precision(reason="f32r norm"):
                        nc.vector.reciprocal(
                            rb[:].rearrange("p a b -> p (a b)"),
                            prb[:].rearrange("p a b -> p (a b)"))
                    state['rb'] = rb

                def fin_head():
                    # UNNORMALIZED attb' = Lsb + gate*Msb (bf16); the
                    # 1/den scale is folded into the yt output multiply
                    attb = att_pool.tile([P, NCH, 512], adt, tag="attb")
                    pgb = aps.tile([P, NCH, 512], f32, tag="mm",
                                   name="pgb")
                    for ch in range(NCH):
                        mm(pgb[:, ch, :], ones_row[:],
                           gate[0:1, k, ch * 512:(ch + 1) * 512])
                    for ch in range(NCH):
                        t2 = cmb.tile([P, 512], f32, tag="t2")
                        nc.vector.tensor_tensor(t2[:], Msb[:, ch, :],
                                                pgb[:, ch, :], OP.mult)
                        with nc.allow_low_precision(reason="bf16 att"):
                            nc.vector.tensor_tensor(attb[:, ch, :], t2[:],
                                                    Lsb[:, ch, :], OP.add)
                    state['attb'] = attb

                def fin_py(ot):
                    py = aps.tile([P, NCH, 512], f32, tag="ll",
                                  name=f"py{ot}")
                    for ch in range(NCH):
                        mm(py[:, ch, :],
                           wot[:, k * C + ot * P:k * C + (ot + 1) * P],
                           state['attb'][:, ch, :])
                    yt = ysb.tile([P, NCH, 512], mybir.dt.bfloat16,
                                  tag="y")
                    with nc.allow_low_precision(reason="bf16 out"):
                        nc.vector.tensor_tensor(
                            yt[:].rearrange("p a b -> p (a b)"),
                            py[:].rearrange("p a b -> p (a b)"),
                            state['rb'][:].rearrange("p a b -> p (a b)"),
                            OP.mult)
                    eng = nc.sync if ot % 2 == 0 else nc.gpsimd
                    eng.dma_start(
                        out=yp[k * C + ot * P:k * C + (ot + 1) * P, :],
                        in_=yt[:].rearrange("p a b -> p (a b)"))

                return [fin_early, fin_head] + \
                    [lambda ot=ot: fin_py(ot) for ot in range(NCT)]

            for k in range(3):
                kh = kh0 if k < 2 else kh1
                voff = 0 if k < 2 else P
                loc_end = 8 if k == 0 else NLOC
                msk_lo = {0: 0, 1: 8, 2: 0}[k]
                # locals first, then memory: the "mm" (memory-AV) psum
                # banks stay free until idx ~16+lag, giving the deferred
                # finalize of the previous slot its psum without ever
                # touching the scores rotation
                js = list(range(loc_end)) + list(range(NLOC, NT))
                Rt = vec.tile([P, THALF], mybir.dt.float16, tag="R")
                Lsb = att_pool.tile([P, NCH, 512], f32, tag="Lsb")
                Msb = att_pool.tile([P, NCH, 512], f32, tag="Msb")
                qrhs = qsb[:, k, :]
                pacc = {}
                Et = {}
                pend = []

                def emit_av(j, k=k, voff=voff, loc_end=loc_end, pacc=pacc,
                            Et=Et):
                    spn = min(P, S - j * P)
                    E2 = Et.pop(j)
                    reg = 'l' if j < NLOC else 'm'
                    first = j == 0 or j == NLOC
                    last = j == loc_end - 1 or j == NT - 1
                    if first:
                        pacc[reg] = aps.tile(
                            [P, NCH, 512], f32,
                            tag=("ll" if reg == 'l' else "mm"),
                            name=f"pacc{reg}")
                    for ch in range(NCH):
                        mm(pacc[reg][:, ch, :], vh[:spn, j, voff:voff + P],
                           E2[:spn, ch * 512:(ch + 1) * 512],
                           start=first, stop=last)

                for idx, j in enumerate(js):
                    # finalize pieces: den+recip at idx 2 (after the new
                    # slot's scores pipeline is rolling — emitting it at
                    # slot end stalled the PE behind the Msb copy), attb
                    # chain at 8, out-proj pieces at 10,12,... so each
                    # py matmul finds attb/yt ready
                    if finalize_prev and (idx == 2 or
                                          (idx >= 8 and idx % 2 == 0)):
                        finalize_prev.pop(0)()
                    spn = min(P, S - j * P)
                    masked = msk_lo <= j < loc_end
                    ps = aps.tile([P, NCH, 512], f32, tag="sc", bufs=2)
                    for ch in range(NCH):
                        mm(ps[:spn, ch, :], kh[:, j * P:j * P + spn],
                           qrhs[:, ch * 512:(ch + 1) * 512],
                           start=True, stop=not masked)
                    if masked:
                        # additive causal bias: -M * count(t < s) via
                        # (-M*lower_tri) @ shifted 0/1 band; per-core
                        # half-offset is baked into the band data
                        bsl = 1 if k == 2 else 0
                        u0 = 1920 - 128 * (j - msk_lo)
                        for ch in range(NCH):
                            mm(ps[:spn, ch, :], trineg[:, :spn],
                               bandm[:, bsl, u0 + ch * 512:
                                     u0 + (ch + 1) * 512],
                               start=False, stop=True)
                    E2 = ep.tile([P, THALF], f16, tag="E")
                    nc.scalar.activation(E2[:spn], ps[:spn].rearrange(
                        "p a b -> p (a b)"), AF.Exp, scale=DSCALE)
                    if idx == 0:
                        nc.vector.tensor_copy(out=Rt[:, :], in_=E2[:, :])
                    else:
                        nc.vector.tensor_tensor(Rt[:spn, :], Rt[:spn, :],
                                                E2[:spn, :], OP.add)
                    Et[j] = E2
                    pend.append(j)
                    # lag 12 early (lets the deferred finalize borrow the
                    # AV psum tags), tapering to 4 so the end-of-slot AV
                    # flush stays small (no ACT bubble)
                    if len(pend) > 11:
                        emit_av(pend.pop(0))
                    if idx >= 26 and len(pend) > 4:
                        emit_av(pend.pop(0))
                for j in pend:
                    emit_av(j)
                pend = []
                if finalize_prev:
                    for piece in finalize_prev:
                        piece()
                nc.vector.tensor_copy(out=Lsb[:], in_=pacc.pop('l')[:])
                nc.vector.tensor_copy(out=Msb[:], in_=pacc.pop('m')[:])
                finalize_prev = make_finalize(k, Rt, Lsb, Msb)
            for piece in finalize_prev:
                piece()
    nc.compile()
    return nc


def make_in_maps(x, forward_memory, reverse_memory, ctrl, Wq, Wk, Wv, Wo,
                 Wc, Wg, bg):
    f = np.float32
    import ml_dtypes
    bf = ml_dtypes.bfloat16
    # causal bias constants: bias[i,c] = -M * #{k: c-sh < k <= i}
    # = tri_neg.T @ band, band[k,u] = (u < k + 1920 - 1024*half)
    MNEG = 1.0e4
    kk = np.arange(P).reshape(P, 1)
    ii = np.arange(P).reshape(1, P)
    trineg = (-MNEG * (ii >= kk)).astype(np.float16)       # lhsT [k, i]
    uu = np.arange(3072).reshape(1, 3072)
    # host-side tiny precomputes (fused-query gate + ctrl bias)
    cq = (Wc @ ctrl).astype(f)                   # [C] = Wc @ ctrl
    wf_full = (Wg @ Wq).astype(f)                # [H, C] rows = Wg_h @ Wq
    gb_full = (Wg @ cq + bg).astype(f)           # [H]
    in_maps = []
    for core in range(8):
        b, g = core // 4, core % 4
        units = slot_units(g)
        hp, hs, _ = GROUP_MAP[g]
        kv = np.concatenate(
            [x[b], forward_memory[b], reverse_memory[b]], axis=0)
        kvT = np.ascontiguousarray(kv.T, dtype=f)
        xqT = np.concatenate(
            [np.ascontiguousarray(x[b, h2 * THALF:(h2 + 1) * THALF, :].T)
             for (_, h2) in units], axis=1)
        wqT = np.concatenate(
            [np.ascontiguousarray(Wq[h * P:(h + 1) * P, :].T)
             for (h, _) in units], axis=1)
        wkT0 = np.ascontiguousarray(Wk[hp * P:(hp + 1) * P, :].T)
        wkT1 = np.ascontiguousarray(Wk[hs * P:(hs + 1) * P, :].T)
        wvT2 = np.concatenate(
            [np.ascontiguousarray(Wv[h * P:(h + 1) * P, :].T)
             for h in (hp, hs)], axis=1)
        woT = np.concatenate(
            [np.ascontiguousarray(Wo[:, h * P:(h + 1) * P].T)
             for (h, _) in units], axis=1)
        qbs = np.stack([cq[h * P:(h + 1) * P] for (h, _) in units],
                       axis=1)                                       # [P,3]
        # per-slot gate rows, computed on host (tiny: x @ (Wg_h@Wq) + gb)
        gateh = np.empty((1, 3 * THALF), dtype=f)
        for kslot, (h, half) in enumerate(units):
            logit = x[b, half * THALF:(half + 1) * THALF, :] @ \
                wf_full[h, :] + gb_full[h]
            gateh[0, kslot * THALF:(kslot + 1) * THALF] = \
                1.0 / (1.0 + np.exp(-logit))
        half2 = units[2][1]
        band01 = (uu < kk + 1920).astype(np.float16)
        band2 = (uu < kk + 1920 - THALF * half2).astype(np.float16)
        bandm = np.concatenate([band01, band2], axis=1)      # [P, 2*3072]
        in_maps.append({
            "kvT": kvT.astype(bf), "xqT": np.ascontiguousarray(
                xqT, dtype=f).astype(bf),
            "wqT": np.ascontiguousarray(wqT, dtype=f).astype(bf),
            "wkT0": wkT0.astype(bf), "wkT1": wkT1.astype(bf),
            "wvT2": np.ascontiguousarray(wvT2, dtype=f).astype(bf),
            "woT": np.ascontiguousarray(woT, dtype=f),
            "qbs": np.ascontiguousarray(qbs, dtype=f),
            "gateh": gateh,
            "bandm": bandm, "trineg": trineg,
            "ones_r": np.ones((1, P), dtype=f),
        })
    return in_maps


def unshard(results):
    y = np.zeros((B, T, C), dtype=np.float32)
    for core in range(8):
        b, g = core // 4, core % 4
        ypc = results[core]["yp"].astype(np.float32)
        for kslot, (_, half) in enumerate(slot_units(g)):
            y[b, half * THALF:(half + 1) * THALF, :] += \
                ypc[kslot * C:(kslot + 1) * C, :].T
    return y


_nc_cache = {}


def _get_nc(use_f32r=True, debug=False, att_bf16=True):
    key = (use_f32r, debug, att_bf16)
    if key not in _nc_cache:
        _nc_cache[key] = build_nc(use_f32r, debug, att_bf16)
    return _nc_cache[key]


def kernel(**inputs):
    return kernel_ex(**inputs)[0]


def kernel_ex(trace=False, trace_cores=None, use_f32r=True, debug=False,
              att_bf16=True, **inputs):
    from concourse.bass_utils import run_bass_kernel_spmd

    np_inputs = {k: np.asarray(v) for k, v in inputs.items()}
    in_maps = make_in_maps(**np_inputs)
    nc = _get_nc(use_f32r, debug, att_bf16)
    res = run_bass_kernel_spmd(nc, in_maps, list(range(8)), trace=trace,
                               trace_cores=trace_cores)
    return unshard(res.results), res
